# revision 63
# baseline (speedup 1.0000x reference)
"""Multi-head attention on 8 Trainium2 NeuronCores.

Problem: B=2, L=2048, D=1024, N=16 heads, H=64.
Sharding: core i -> batch (i // 4), heads [4*(i%4), 4*(i%4)+4).
Each core: QKV projections for its 4 heads, full-seq attention,
partial output projection. Host sums the 4 partial projections per batch.

Design notes:
- Heads are processed in pairs (2p, 2p+1): head 2p in SBUF partitions
  0-63, head 2p+1 in 64-127 (the natural QKV projection layout).
- Logits (K=64 contraction) run as two concurrent 64x128 row tiles via
  tile_position (0,0)/(64,0) -- no zero padding, ~2x the padded rate.
  qt/kt are bf16: f32r moving operands run at half rate in tiled mode.
- One exp instruction covers a head pair [128, 2, 512] (N=1024 on ACT).
- PV keeps the ones-column trick (M=65) so softmax norms are free.
- The out-projection packs a head pair into the full K=128 contraction
  (rows 0-63 head 2p, 64-127 head 2p+1, both real data): one matmul per
  pair sums both heads at full rate.
- Phase 1 is pipelined for an early ACT start: x tensors load panel-major
  across both HW DMA queues (the ACT queue carries only pre-first-exp
  data), dummy matmuls warm the PE HAM clock during the DMA wait, K cell
  (0,0) + Q(0,0) unblock the first logits, and the remaining K/Q/V cells
  interleave into the first pump steps under issue-order deadlines
  (every tile writer must precede its reader in program order, since
  logits are issued PREF steps ahead).
- The exp stream is the pacing engine (~1.1us per [128,1024] exp, 128
  total); everything else hides under it except the ramp (~29us) and the
  tail (normalize chain + last out-proj + Tile end barrier, ~30us).
"""

import numpy as np

B, L, D = 2, 2048, 1024
NHEADS, HDIM = 16, 64
NCORES = 8
HPC = 4  # heads per core
NP = 2  # head pairs per core
E = HPC * HDIM  # 256
DCH = D // 128  # 8 d-chunks
TCH = L // 128  # 16 t-chunks
FB = 512  # f-block size in attention phase
NFB = L // FB  # 4
VW = HDIM + 1  # V' width per head (64 cols V + 1 ones col)

_CACHED_NC = None


def _build_nc():
    import concourse.mybir as mybir
    from concourse import bacc
    from concourse.tile import TileContext
    from collections import deque

    f32 = mybir.dt.float32
    bf16 = mybir.dt.bfloat16
    EXP = mybir.ActivationFunctionType.Exp

    nc = bacc.Bacc("TRN2", target_bir_lowering=False, num_devices=NCORES)

    # xq/xv arrive panel-major from the host ([panel, partition, chunk, 512])
    # so each 1MB panel DMA moves 8KB-contiguous runs per partition
    xq = nc.declare_dram_parameter("xq", [NFB, 128, DCH, 512], bf16, isOutput=False)
    xk = nc.declare_dram_parameter("xk", [NFB, 128, DCH, 512], bf16, isOutput=False)
    xv = nc.declare_dram_parameter("xv", [NFB, 128, DCH, 512], bf16, isOutput=False)
    wq = nc.declare_dram_parameter("wq", [D, E], bf16, isOutput=False)
    wk = nc.declare_dram_parameter("wk", [D, E], bf16, isOutput=False)
    wv = nc.declare_dram_parameter("wv", [D, E], bf16, isOutput=False)
    wo = nc.declare_dram_parameter("wo", [E, D], bf16, isOutput=False)
    out = nc.declare_dram_parameter("out", [L, D], bf16, isOutput=True)

    with TileContext(nc) as tc:
        with tc.tile_pool(name="persist", bufs=1) as cpool:
            # --- persistent SBUF tensors ---
            wq_sb = cpool.tile([128, DCH, E], bf16, tag="wq")
            wk_sb = cpool.tile([128, DCH, E], bf16, tag="wk")
            wv_sb = cpool.tile([128, DCH, E], bf16, tag="wv")
            # wo paged by pair: rows 0-63 head 2p, 64-127 head 2p+1
            wo_sb = cpool.tile([128, NP, D], bf16, tag="wo")
            qt_sb = cpool.tile([128, NP, L], bf16, tag="qt")
            kt_sb = cpool.tile([128, NP, L], bf16, tag="kt")
            v_sb = cpool.tile([128, TCH, HPC * VW], bf16, tag="v")
            ont = cpool.tile([128, NP, L], bf16, tag="ont")
            ones_f32 = cpool.tile([1, HDIM], f32, tag="ones")
            warm_sb = cpool.tile([128, 512], bf16, tag="warm")

            with tc.tile_pool(name="xp", bufs=1) as xpool:

                # ramp loads split across the two HW DMA queues (SP + ACT;
                # ACT is idle during the ramp), ordered so the data the
                # pipeline needs first arrives first: xk (split across both
                # queues), then the f-block-0 panel of xq, then xv panels,
                # then the remaining xq panels.
                xtk = xpool.tile([128, NFB, DCH, 512], bf16, tag="xk")
                xtq = xpool.tile([128, NFB, DCH, 512], bf16, tag="xq")
                xtv = xpool.tile([128, NFB, DCH, 512], bf16, tag="xv")
                # scalar (ACT) queue carries ONLY data needed before the
                # first exp — anything later would block the exp stream in
                # the ACT FIFO. Everything else rides the sync queue.
                # All x tensors are panel-major; panels land in the order
                # the pipeline consumes them.
                nc.sync.dma_start(
                    out=wk_sb[:], in_=wk.rearrange("(c p) e -> p c e", p=128)
                )
                nc.scalar.dma_start(
                    out=wq_sb[:], in_=wq.rearrange("(c p) e -> p c e", p=128)
                )
                nc.sync.dma_start(out=xtk[:, 0, :, :], in_=xk[0, :, :, :])
                nc.scalar.dma_start(out=xtk[:, 1, :, :], in_=xk[1, :, :, :])
                nc.scalar.dma_start(out=xtq[:, 0, :, :], in_=xq[0, :, :, :])
                nc.sync.dma_start(out=xtk[:, 2, :, :], in_=xk[2, :, :, :])
                nc.scalar.dma_start(out=xtk[:, 3, :, :], in_=xk[3, :, :, :])
                nc.scalar.dma_start(out=xtq[:, 1, :, :], in_=xq[1, :, :, :])
                nc.sync.dma_start(
                    out=wv_sb[:], in_=wv.rearrange("(c p) e -> p c e", p=128)
                )
                # xv in 4 column panels (one per 4 v-passes)
                for vp2 in range(4):
                    nc.sync.dma_start(
                        out=xtv[:, vp2, :, :], in_=xv[vp2, :, :, :]
                    )
                for lb in range(2, 4):
                    nc.sync.dma_start(
                        out=xtq[:, lb, :, :], in_=xq[lb, :, :, :]
                    )
                nc.sync.dma_start(
                    out=wo_sb[:], in_=wo.rearrange("(p x) d -> x p d", x=128)
                )
                nc.vector.memset(
                    v_sb[:].rearrange("p t (n c) -> p t n c", n=HPC)[
                        :, :, :, HDIM : HDIM + 1
                    ],
                    1.0,
                )
                nc.vector.memset(ones_f32[:], 1.0)
                nc.vector.memset(warm_sb[:], 0.0)

                def xk_sl(d, lb):
                    return xtk[:, lb, d, :]

                def xq_sl(d, lb):
                    return xtq[:, lb, d, :]

                def kproj_cells(pool, cells, w_sb, x_sl, dst):
                    pss = [
                        pool.tile(
                            [128, 512], f32, tag="ph1", name=f"kc_{e}_{lb}"
                        )
                        for (e, lb) in cells
                    ]
                    for d in range(DCH):
                        for ps, (e, lb) in zip(pss, cells):
                            nc.tensor.matmul(
                                ps[:],
                                w_sb[:, d, e * 128 : (e + 1) * 128],
                                x_sl(d, lb),
                                start=(d == 0),
                                stop=(d == DCH - 1),
                            )
                    for ps, (e, lb) in zip(pss, cells):
                        nc.vector.tensor_copy(
                            out=dst[:, e, lb * 512 : (lb + 1) * 512], in_=ps[:]
                        )

                # ---- Phase 2: ACT-paced global pump over (fb, pair, t) ----
                PREF = 18

                steps = [
                    (fb, p, t)
                    for fb in range(NFB)
                    for p in range(NP)
                    for t in range(TCH)
                ]

                with (
                    tc.tile_pool(name="psL", bufs=2, space="PSUM") as psL,
                    tc.tile_pool(name="ep", bufs=PREF) as epool,
                    tc.tile_pool(name="rp", bufs=2) as rpool,
                    tc.tile_pool(name="bp", bufs=1) as bpool,
                    tc.tile_pool(name="op", bufs=3) as opool,
                ):
                    cursor = [0]
                    pslq = deque()
                    etq = deque()

                    def pump_logits():
                        k = cursor[0]
                        if k >= len(steps):
                            return
                        cursor[0] += 1
                        fb, p, t = steps[k]
                        f0 = fb * FB
                        psl = psL.tile(
                            [128, 2, FB], f32, tag="psl", name=f"psl_{k}"
                        )
                        for j in range(2):
                            lo = j * 64
                            nc.tensor.matmul(
                                psl[:, j, :],
                                kt_sb[lo : lo + 64, p, t * 128 : (t + 1) * 128],
                                qt_sb[lo : lo + 64, p, f0 : f0 + FB],
                                start=True,
                                stop=True,
                            )
                        pslq.append((k, psl))

                    def pump_exp():
                        if not pslq:
                            return
                        k, psl = pslq.popleft()
                        et = epool.tile(
                            [128, 2, FB], bf16, tag="e", name=f"et_{k}"
                        )
                        nc.scalar.activation(
                            et[:].rearrange("p a b -> p (a b)"),
                            psl[:].rearrange("p a b -> p (a b)"),
                            EXP,
                        )
                        etq.append(et)

                    def pump():
                        pump_logits()
                        pump_exp()

                    pending = []
                    psW_holder = [None]
                    psP_cm_holder = [None]

                    def outproj_group(lc, db, drain=0):
                        if psW_holder[0] is None:
                            # phase-1 residual pool is drained by now;
                            # trade its banks for the out-proj pool
                            psP_cm_holder[0].__exit__(None, None, None)
                            psP_cm_holder[0] = None
                            cm = tc.tile_pool(name="psW", bufs=2, space="PSUM")
                            psW_holder[0] = (cm, cm.__enter__())
                        # pair pages pack 2 heads into the K=128 contraction
                        # (rows 0-63 head 2p, 64-127 head 2p+1): one matmul
                        # per pair sums both heads at full rate
                        ps = psW_holder[0][1].tile(
                            [128, FB], f32, tag="w", name=f"ps3_{lc}_{db}"
                        )
                        for p_ in range(NP):
                            nc.tensor.matmul(
                                ps[:],
                                ont[:, p_, lc * 128 : (lc + 1) * 128],
                                wo_sb[:, p_, db * FB : (db + 1) * FB],
                                start=(p_ == 0),
                                stop=(p_ == NP - 1),
                            )
                        ot = opool.tile(
                            [128, FB], bf16, tag="o", name=f"ot_{lc}_{db}"
                        )
                        if drain % 2 == 1:  # ACT is idle after the last exp
                            nc.scalar.copy(out=ot[:], in_=ps[:])
                        else:
                            nc.vector.tensor_copy(out=ot[:], in_=ps[:])
                        eng = nc.scalar if drain and drain % 2 == 0 else nc.sync
                        eng.dma_start(
                            out=out[
                                lc * 128 : (lc + 1) * 128,
                                db * FB : (db + 1) * FB,
                            ],
                            in_=ot[:],
                        )

                    # psO outlives psP/psW; open it first (pools are a stack)
                    psO_cm = tc.tile_pool(name="psO", bufs=2, space="PSUM")
                    psO = psO_cm.__enter__()
                    # Phase-1 residual work (K pair 1, Q cells, V passes)
                    # interleaves into the first pump steps on pool psP.
                    psP_cm = tc.tile_pool(name="psP", bufs=2, space="PSUM")
                    psP = psP_cm.__enter__()
                    psP_cm_holder[0] = psP_cm

                    # HAM warmup: dummy matmuls occupy the xk DMA wait so
                    # the PE is at full clock when real work lands
                    wps = psP.tile([128, 512], f32, tag="ph1", name="warm_ps")
                    for _ in range(10):
                        nc.tensor.matmul(
                            wps[:],
                            warm_sb[:, 0:128],
                            warm_sb[:],
                            start=True,
                            stop=True,
                        )
                    # K/Q projection cells pipelined with the pump: logits
                    # for t-chunks [4lb, 4lb+4) need K cell (0, lb), so the
                    # exp stream starts right after cell (0,0) + Q(0,0).
                    kproj_cells(psP, [(0, 0)], wk_sb, xk_sl, kt_sb)
                    kproj_cells(psP, [(0, 0)], wq_sb, xq_sl, qt_sb)
                    pump()
                    pump()
                    pump()
                    pump()
                    kproj_cells(psP, [(0, 1)], wk_sb, xk_sl, kt_sb)
                    pump()
                    pump()
                    pump()
                    pump()
                    kproj_cells(psP, [(0, 2)], wk_sb, xk_sl, kt_sb)
                    pump()
                    pump()
                    pump()
                    pump()
                    kproj_cells(psP, [(0, 3)], wk_sb, xk_sl, kt_sb)
                    # first cells of K pair 1 / Q pair 1 must be issued
                    # before the prime pumps reach pair-1 logits (cursor can
                    # pass 16); the rest spread into the first steps as
                    # deadline-scheduled work items
                    kproj_cells(psP, [(1, 0)], wk_sb, xk_sl, kt_sb)
                    kproj_cells(psP, [(1, 0)], wq_sb, xq_sl, qt_sb)
                    kproj_cells(psP, [(1, 1)], wk_sb, xk_sl, kt_sb)
                    while cursor[0] < PREF:
                        pump()

                    # remaining Q cells, split into d-halves (one half per
                    # step keeps psl delivery timely)
                    qstate = [None]

                    def q_item(item):
                        (e, lb), half = item
                        if half == 0:
                            qstate[0] = psP.tile(
                                [128, 512], f32, tag="ph1", name=f"qc_{e}_{lb}"
                            )
                        ps = qstate[0]
                        for d in range(half * 4, half * 4 + 4):
                            nc.tensor.matmul(
                                ps[:],
                                wq_sb[:, d, e * 128 : (e + 1) * 128],
                                xq_sl(d, lb),
                                start=(d == 0),
                                stop=(d == DCH - 1),
                            )
                        if half == 1:
                            nc.vector.tensor_copy(
                                out=qt_sb[:, e, lb * 512 : (lb + 1) * 512],
                                in_=ps[:],
                            )

                    # late Q cells: the first pair stays tight to its
                    # deadline (steps 16-19); the second pair moves into the
                    # idle back half of fb0p1 (psP is still open there)
                    qsched = {
                        16: ((0, 2), 0),
                        17: ((0, 2), 1),
                        18: ((1, 2), 0),
                        19: ((1, 2), 1),
                        24: ((0, 3), 0),
                        26: ((0, 3), 1),
                        28: ((1, 3), 0),
                        30: ((1, 3), 1),
                    }

                    # V projection one t-chunk per step (single psP slot)
                    def v_item(t):
                        ps = psP.tile([128, E], f32, tag="ph1", name=f"v_{t}")
                        for d in range(DCH):
                            nc.tensor.matmul(
                                ps[:],
                                xtv[:, t // 4, d, (t % 4) * 128 : (t % 4 + 1) * 128],
                                wv_sb[:, d, :],
                                start=(d == 0),
                                stop=(d == DCH - 1),
                            )
                        nc.vector.tensor_copy(
                            out=v_sb[:, t, :].rearrange(
                                "p (n c) -> p n c", n=HPC
                            )[:, :, 0:HDIM],
                            in_=ps[:].rearrange("p (n c) -> p n c", n=HPC),
                        )

                    vitems = deque(range(TCH))
                    # K pair-1 / Q cells with early-step deadlines
                    kq_items = deque(
                        [
                            ("k", (1, 2), None),
                            ("k", (1, 3), None),
                            ("q", (0, 1), 0),
                            ("q", (0, 1), 1),
                            ("q", (1, 1), 0),
                            ("q", (1, 1), 1),
                        ]
                    )

                    psoAB = [None, None]

                    for si, (fb, p, t) in enumerate(steps):
                        f0 = fb * FB
                        first_block = fb == 0 and p == 0
                        if t == 0:
                            psoAB[0] = psO.tile(
                                [VW, FB], f32, tag="pso", name=f"psoA_{fb}_{p}"
                            )
                            psoAB[1] = psO.tile(
                                [VW, FB], f32, tag="pso", name=f"psoB_{fb}_{p}"
                            )
                        if si % 2 == 0:
                            pump_logits()
                            pump_logits()
                        pump_exp()
                        if first_block and vitems:
                            v_item(vitems.popleft())
                        if first_block and t % 2 == 1 and kq_items:
                            kind, cell, half = kq_items.popleft()
                            if kind == "k":
                                kproj_cells(psP, [cell], wk_sb, xk_sl, kt_sb)
                            else:
                                q_item((cell, half))
                        if si in qsched:
                            q_item(qsched[si])
                        et = etq.popleft()
                        for j in range(2):
                            h = 2 * p + j
                            nc.tensor.matmul(
                                psoAB[j][:],
                                v_sb[:, t, h * VW : (h + 1) * VW],
                                et[:, j, :],
                                start=(t == 0),
                                stop=(t == TCH - 1),
                            )
                        if t % 4 == 2 and pending:
                            outproj_group(*pending.pop(0))

                        if t == TCH - 1:
                            # normalize: O.T = O_un.T * (1/norm); both heads'
                            # norm rows share one DMA round trip via a fused
                            # stage tile [VW, 2, FB]
                            last_block = si == len(steps) - 1
                            stage = rpool.tile(
                                [VW, 2, FB],
                                f32,
                                tag="st",
                                name=f"stage_{fb}_{p}",
                            )
                            nc.vector.tensor_copy(
                                out=stage[:, 0, :], in_=psoAB[0][:]
                            )
                            if last_block:
                                nc.scalar.copy(
                                    out=stage[:, 1, :], in_=psoAB[1][:]
                                )
                            else:
                                nc.vector.tensor_copy(
                                    out=stage[:, 1, :], in_=psoAB[1][:]
                                )
                            if last_block:
                                # keep HAM warm through the normalize chain
                                wps2 = psL.tile(
                                    [128, 2, FB], f32, tag="psl", name="warm2"
                                )
                                for _ in range(12):
                                    nc.tensor.matmul(
                                        wps2[:, 0, :],
                                        warm_sb[:, 0:128],
                                        warm_sb[:],
                                        start=True,
                                        stop=True,
                                    )
                            dmae = nc.scalar if last_block else nc.sync
                            rsq = rpool.tile([32, FB // 16], f32, tag="rsq")
                            dmae.dma_start(
                                out=rsq[:],
                                in_=stage[HDIM : HDIM + 1, :, :],
                            )
                            rsf = rpool.tile([32, FB // 16], f32, tag="rsf")
                            with nc.allow_low_precision(
                                reason="softmax recip"
                            ):
                                nc.vector.reciprocal(out=rsf[:], in_=rsq[:])
                            rt2 = rpool.tile([1, 2, FB], f32, tag="rt2")
                            dmae.dma_start(out=rt2[:], in_=rsf[:])
                            for j in range(2):
                                if last_block:
                                    # matmul broadcast: psO slots are free to
                                    # hold (no next block), and it skips the
                                    # slow gpsimd+drain on the critical tail
                                    bt = psO.tile(
                                        [64, FB],
                                        f32,
                                        tag="pso",
                                        name=f"bt_{fb}_{p}_{j}",
                                    )
                                    nc.tensor.matmul(
                                        bt[:],
                                        ones_f32[0:1, :],
                                        rt2[0:1, j, :],
                                        start=True,
                                        stop=True,
                                    )
                                else:
                                    bt = bpool.tile(
                                        [64, FB], f32, tag=f"b{j}"
                                    )
                                    nc.gpsimd.partition_broadcast(
                                        bt[:], rt2[0:1, j, :], channels=64
                                    )
                                if j == 0:
                                    nc.vector.tensor_mul(
                                        out=ont[0:HDIM, p, f0 : f0 + FB],
                                        in0=stage[0:HDIM, 0, :],
                                        in1=bt[:],
                                    )
                                else:
                                    tmp = bpool.tile([64, FB], bf16, tag="tmp")
                                    nc.vector.tensor_mul(
                                        out=tmp[:],
                                        in0=stage[0:HDIM, 1, :],
                                        in1=bt[:],
                                    )
                                    dmae.dma_start(
                                        out=ont[64:128, p, f0 : f0 + FB],
                                        in_=tmp[:],
                                    )
                            if p == NP - 1:
                                pending += [
                                    (lc, db)
                                    for lc in range(
                                        fb * (FB // 128),
                                        (fb + 1) * (FB // 128),
                                    )
                                    for db in range(D // FB)
                                ]

                    # drain the last f-block's out-projection
                    for gi, (lc, db) in enumerate(pending):
                        outproj_group(lc, db, drain=gi + 1)
                    psW_holder[0][0].__exit__(None, None, None)
                    psO_cm.__exit__(None, None, None)

    nc.compile()
    return nc


def _get_nc():
    global _CACHED_NC
    if _CACHED_NC is None:
        _CACHED_NC = _build_nc()
    return _CACHED_NC


def _make_in_maps(query_input, key_input, value_input, Wq, Wk, Wv, Wo):
    import ml_dtypes

    bf16 = ml_dtypes.bfloat16
    scale = np.float32(HDIM) ** np.float32(-0.5)

    def panel_tile(x):
        # [D, L] -> [panel, partition, chunk, 512]
        return np.ascontiguousarray(
            x.T.reshape(DCH, 128, NFB, 512).transpose(2, 1, 0, 3)
        ).astype(bf16)

    xT = {}
    for b in range(B):
        xT[("q", b)] = panel_tile(query_input[b])
        xT[("k", b)] = panel_tile(key_input[b])
        xT[("v", b)] = panel_tile(value_input[b])

    in_maps = []
    for core in range(NCORES):
        b = core // 4
        g = core % 4
        hs = slice(g * HPC, (g + 1) * HPC)
        in_maps.append(
            {
                "xq": xT[("q", b)],
                "xk": xT[("k", b)],
                "xv": xT[("v", b)],
                "wq": np.ascontiguousarray(
                    (Wq[:, hs, :] * scale).reshape(D, E)
                ).astype(bf16),
                "wk": np.ascontiguousarray(Wk[:, hs, :].reshape(D, E)).astype(bf16),
                "wv": np.ascontiguousarray(Wv[:, hs, :].reshape(D, E)).astype(bf16),
                "wo": np.ascontiguousarray(Wo[hs].reshape(E, D)).astype(bf16),
            }
        )
    return in_maps


def _combine(results):
    out = np.empty((B, L, D), dtype=np.float32)
    for b in range(B):
        acc = results[b * 4]["out"].astype(np.float32)
        for g in range(1, 4):
            acc = acc + results[b * 4 + g]["out"]
        out[b] = acc
    return out


def kernel(query_input, key_input, value_input, Wq, Wk, Wv, Wo):
    from concourse.bass_utils import run_bass_kernel_spmd

    nc = _get_nc()
    in_maps = _make_in_maps(query_input, key_input, value_input, Wq, Wk, Wv, Wo)
    res = run_bass_kernel_spmd(nc, in_maps, core_ids=list(range(NCORES)))
    return _combine(res.results)


if __name__ == "__main__":
    rng = np.random.default_rng(0)
    inputs = {
        "query_input": rng.standard_normal((B, L, D), dtype=np.float32),
        "key_input": rng.standard_normal((B, L, D), dtype=np.float32),
        "value_input": rng.standard_normal((B, L, D), dtype=np.float32),
        "Wq": rng.standard_normal((D, NHEADS, HDIM), dtype=np.float32) * 0.03,
        "Wk": rng.standard_normal((D, NHEADS, HDIM), dtype=np.float32) * 0.03,
        "Wv": rng.standard_normal((D, NHEADS, HDIM), dtype=np.float32) * 0.03,
        "Wo": rng.standard_normal((NHEADS, HDIM, D), dtype=np.float32) * 0.03,
    }
    out = kernel(**inputs)
    print("kernel output", out.shape, out.dtype, float(np.abs(out).mean()))


# revision 67
# speedup vs baseline: 1.0404x; 1.0404x over previous
"""Multi-head attention on 8 Trainium2 NeuronCores.

Problem: B=2, L=2048, D=1024, N=16 heads, H=64.
Sharding: core i -> batch (i // 4), heads [4*(i%4), 4*(i%4)+4).
Each core: QKV projections for its 4 heads, full-seq attention,
partial output projection. Host sums the 4 partial projections per batch.

Design notes:
- Heads are processed in pairs (2p, 2p+1): head 2p in SBUF partitions
  0-63, head 2p+1 in 64-127 (the natural QKV projection layout).
- Logits (K=64 contraction) run as two concurrent 64x128 row tiles via
  tile_position (0,0)/(64,0) -- no zero padding, ~2x the padded rate.
  qt/kt are bf16: f32r moving operands run at half rate in tiled mode.
- One exp instruction covers a head pair [128, 2, 512] (N=1024 on ACT).
- PV keeps the ones-column trick (M=65) so softmax norms are free.
- The out-projection packs a head pair into the full K=128 contraction
  (rows 0-63 head 2p, 64-127 head 2p+1, both real data): one matmul per
  pair sums both heads at full rate.
- Phase 1 is pipelined for an early ACT start: x tensors load panel-major
  across both HW DMA queues (the ACT queue carries only pre-first-exp
  data), dummy matmuls warm the PE HAM clock during the DMA wait, K cell
  (0,0) + Q(0,0) unblock the first logits, and the remaining K/Q/V cells
  interleave into the first pump steps under issue-order deadlines
  (every tile writer must precede its reader in program order, since
  logits are issued PREF steps ahead).
- The exp stream is the pacing engine (~1.1us per [128,1024] exp, 128
  total); everything else hides under it except the ramp (~29us) and the
  tail (normalize chain + last out-proj + Tile end barrier, ~30us).
"""

import numpy as np

B, L, D = 2, 2048, 1024
NHEADS, HDIM = 16, 64
NCORES = 8
HPC = 4  # heads per core
NP = 2  # head pairs per core
E = HPC * HDIM  # 256
DCH = D // 128  # 8 d-chunks
TCH = L // 128  # 16 t-chunks
FB = 512  # f-block size in attention phase
NFB = L // FB  # 4
VW = HDIM + 1  # V' width per head (64 cols V + 1 ones col)

_CACHED_NC = None


def _build_nc():
    import concourse.mybir as mybir
    from concourse import bacc
    from concourse.tile import TileContext
    from collections import deque

    f32 = mybir.dt.float32
    bf16 = mybir.dt.bfloat16
    EXP = mybir.ActivationFunctionType.Exp

    nc = bacc.Bacc("TRN2", target_bir_lowering=False, num_devices=NCORES)

    # xq/xv arrive panel-major from the host ([panel, partition, chunk, 512])
    # so each 1MB panel DMA moves 8KB-contiguous runs per partition
    xq = nc.declare_dram_parameter("xq", [NFB, 128, DCH, 512], bf16, isOutput=False)
    xk = nc.declare_dram_parameter("xk", [NFB, 128, DCH, 512], bf16, isOutput=False)
    xv = nc.declare_dram_parameter("xv", [NFB, 128, DCH, 512], bf16, isOutput=False)
    wq = nc.declare_dram_parameter("wq", [D, E], bf16, isOutput=False)
    wk = nc.declare_dram_parameter("wk", [D, E], bf16, isOutput=False)
    wv = nc.declare_dram_parameter("wv", [D, E], bf16, isOutput=False)
    wo = nc.declare_dram_parameter("wo", [E, D], bf16, isOutput=False)
    out = nc.declare_dram_parameter("out", [L, D], bf16, isOutput=True)

    with TileContext(nc) as tc:
        with tc.tile_pool(name="persist", bufs=1) as cpool:
            # --- persistent SBUF tensors ---
            wq_sb = cpool.tile([128, DCH, E], bf16, tag="wq")
            wk_sb = cpool.tile([128, DCH, E], bf16, tag="wk")
            wv_sb = cpool.tile([128, DCH, E], bf16, tag="wv")
            # wo paged by pair: rows 0-63 head 2p, 64-127 head 2p+1
            wo_sb = cpool.tile([128, NP, D], bf16, tag="wo")
            qt_sb = cpool.tile([128, NP, L], bf16, tag="qt")
            kt_sb = cpool.tile([128, NP, L], bf16, tag="kt")
            v_sb = cpool.tile([128, TCH, HPC * VW], bf16, tag="v")
            ont = cpool.tile([128, NP, L], bf16, tag="ont")
            ones_f32 = cpool.tile([1, HDIM], f32, tag="ones")
            warm_sb = cpool.tile([128, 512], bf16, tag="warm")

            with tc.tile_pool(name="xp", bufs=1) as xpool:

                # ramp loads split across the two HW DMA queues (SP + ACT;
                # ACT is idle during the ramp), ordered so the data the
                # pipeline needs first arrives first: xk (split across both
                # queues), then the f-block-0 panel of xq, then xv panels,
                # then the remaining xq panels.
                xtk = xpool.tile([128, NFB, DCH, 512], bf16, tag="xk")
                xtq = xpool.tile([128, NFB, DCH, 512], bf16, tag="xq")
                xtv = xpool.tile([128, NFB, DCH, 512], bf16, tag="xv")
                # scalar (ACT) queue carries ONLY data needed before the
                # first exp — anything later would block the exp stream in
                # the ACT FIFO. Everything else rides the sync queue.
                # All x tensors are panel-major; panels land in the order
                # the pipeline consumes them.
                nc.sync.dma_start(
                    out=wk_sb[:], in_=wk.rearrange("(c p) e -> p c e", p=128)
                )
                nc.scalar.dma_start(
                    out=wq_sb[:], in_=wq.rearrange("(c p) e -> p c e", p=128)
                )
                # the ACT FIFO must drain all scalar-queue DMAs before the
                # first exp can issue -- keep that cargo minimal (2.5MB)
                nc.sync.dma_start(out=xtk[:, 0, :, :], in_=xk[0, :, :, :])
                nc.scalar.dma_start(out=xtq[:, 0, :, :], in_=xq[0, :, :, :])
                nc.scalar.dma_start(out=xtk[:, 1, :, :], in_=xk[1, :, :, :])
                nc.sync.dma_start(out=xtk[:, 2, :, :], in_=xk[2, :, :, :])
                nc.sync.dma_start(out=xtk[:, 3, :, :], in_=xk[3, :, :, :])
                nc.sync.dma_start(out=xtq[:, 1, :, :], in_=xq[1, :, :, :])
                nc.sync.dma_start(
                    out=wv_sb[:], in_=wv.rearrange("(c p) e -> p c e", p=128)
                )
                # xv in 4 column panels (one per 4 v-passes)
                for vp2 in range(4):
                    nc.sync.dma_start(
                        out=xtv[:, vp2, :, :], in_=xv[vp2, :, :, :]
                    )
                for lb in range(2, 4):
                    nc.sync.dma_start(
                        out=xtq[:, lb, :, :], in_=xq[lb, :, :, :]
                    )
                nc.sync.dma_start(
                    out=wo_sb[:], in_=wo.rearrange("(p x) d -> x p d", x=128)
                )
                nc.vector.memset(
                    v_sb[:].rearrange("p t (n c) -> p t n c", n=HPC)[
                        :, :, :, HDIM : HDIM + 1
                    ],
                    1.0,
                )
                nc.vector.memset(ones_f32[:], 1.0)
                nc.vector.memset(warm_sb[:], 0.0)

                def xk_sl(d, lb):
                    return xtk[:, lb, d, :]

                def xq_sl(d, lb):
                    return xtq[:, lb, d, :]

                def kproj_cells(pool, cells, w_sb, x_sl, dst):
                    pss = [
                        pool.tile(
                            [128, 512], f32, tag="ph1", name=f"kc_{e}_{lb}"
                        )
                        for (e, lb) in cells
                    ]
                    for d in range(DCH):
                        for ps, (e, lb) in zip(pss, cells):
                            nc.tensor.matmul(
                                ps[:],
                                w_sb[:, d, e * 128 : (e + 1) * 128],
                                x_sl(d, lb),
                                start=(d == 0),
                                stop=(d == DCH - 1),
                            )
                    for ps, (e, lb) in zip(pss, cells):
                        nc.vector.tensor_copy(
                            out=dst[:, e, lb * 512 : (lb + 1) * 512], in_=ps[:]
                        )

                # ---- Phase 2: ACT-paced global pump over (fb, pair, t) ----
                PREF = 18

                steps = [
                    (fb, p, t)
                    for fb in range(NFB)
                    for p in range(NP)
                    for t in range(TCH)
                ]

                with (
                    tc.tile_pool(name="psL", bufs=2, space="PSUM") as psL,
                    tc.tile_pool(name="ep", bufs=PREF) as epool,
                    tc.tile_pool(name="rp", bufs=2) as rpool,
                    tc.tile_pool(name="bp", bufs=1) as bpool,
                    tc.tile_pool(name="op", bufs=3) as opool,
                ):
                    cursor = [0]
                    pslq = deque()
                    etq = deque()

                    def pump_logits():
                        k = cursor[0]
                        if k >= len(steps):
                            return
                        cursor[0] += 1
                        fb, p, t = steps[k]
                        f0 = fb * FB
                        psl = psL.tile(
                            [128, 2, FB], f32, tag="psl", name=f"psl_{k}"
                        )
                        for j in range(2):
                            lo = j * 64
                            nc.tensor.matmul(
                                psl[:, j, :],
                                kt_sb[lo : lo + 64, p, t * 128 : (t + 1) * 128],
                                qt_sb[lo : lo + 64, p, f0 : f0 + FB],
                                start=True,
                                stop=True,
                            )
                        pslq.append((k, psl))

                    def pump_exp():
                        if not pslq:
                            return
                        k, psl = pslq.popleft()
                        et = epool.tile(
                            [128, 2, FB], bf16, tag="e", name=f"et_{k}"
                        )
                        nc.scalar.activation(
                            et[:].rearrange("p a b -> p (a b)"),
                            psl[:].rearrange("p a b -> p (a b)"),
                            EXP,
                        )
                        etq.append(et)

                    def pump():
                        pump_logits()
                        pump_exp()

                    pending = []
                    psW_holder = [None]
                    psP_cm_holder = [None]

                    def outproj_group(lc, db, drain=0):
                        if psW_holder[0] is None:
                            # phase-1 residual pool is drained by now;
                            # trade its banks for the out-proj pool
                            psP_cm_holder[0].__exit__(None, None, None)
                            psP_cm_holder[0] = None
                            cm = tc.tile_pool(name="psW", bufs=2, space="PSUM")
                            psW_holder[0] = (cm, cm.__enter__())
                        # pair pages pack 2 heads into the K=128 contraction
                        # (rows 0-63 head 2p, 64-127 head 2p+1): one matmul
                        # per pair sums both heads at full rate
                        ps = psW_holder[0][1].tile(
                            [128, FB], f32, tag="w", name=f"ps3_{lc}_{db}"
                        )
                        for p_ in range(NP):
                            nc.tensor.matmul(
                                ps[:],
                                ont[:, p_, lc * 128 : (lc + 1) * 128],
                                wo_sb[:, p_, db * FB : (db + 1) * FB],
                                start=(p_ == 0),
                                stop=(p_ == NP - 1),
                            )
                        ot = opool.tile(
                            [128, FB], bf16, tag="o", name=f"ot_{lc}_{db}"
                        )
                        if drain % 2 == 1:  # ACT is idle after the last exp
                            nc.scalar.copy(out=ot[:], in_=ps[:])
                        else:
                            nc.vector.tensor_copy(out=ot[:], in_=ps[:])
                        eng = nc.scalar if drain and drain % 2 == 0 else nc.sync
                        eng.dma_start(
                            out=out[
                                lc * 128 : (lc + 1) * 128,
                                db * FB : (db + 1) * FB,
                            ],
                            in_=ot[:],
                        )

                    # psO outlives psP/psW; open it first (pools are a stack)
                    psO_cm = tc.tile_pool(name="psO", bufs=2, space="PSUM")
                    psO = psO_cm.__enter__()
                    # Phase-1 residual work (K pair 1, Q cells, V passes)
                    # interleaves into the first pump steps on pool psP.
                    psP_cm = tc.tile_pool(name="psP", bufs=2, space="PSUM")
                    psP = psP_cm.__enter__()
                    psP_cm_holder[0] = psP_cm

                    # HAM warmup: dummy matmuls occupy the xk DMA wait so
                    # the PE is at full clock when real work lands
                    wps = psP.tile([128, 512], f32, tag="ph1", name="warm_ps")
                    for _ in range(10):
                        nc.tensor.matmul(
                            wps[:],
                            warm_sb[:, 0:128],
                            warm_sb[:],
                            start=True,
                            stop=True,
                        )
                    # K/Q projection cells pipelined with the pump: logits
                    # for t-chunks [4lb, 4lb+4) need K cell (0, lb), so the
                    # exp stream starts right after cell (0,0) + Q(0,0).
                    kproj_cells(psP, [(0, 0)], wk_sb, xk_sl, kt_sb)
                    kproj_cells(psP, [(0, 0)], wq_sb, xq_sl, qt_sb)
                    pump()
                    pump()
                    pump()
                    pump()
                    kproj_cells(psP, [(0, 1)], wk_sb, xk_sl, kt_sb)
                    pump()
                    pump()
                    pump()
                    pump()
                    kproj_cells(psP, [(0, 2)], wk_sb, xk_sl, kt_sb)
                    pump()
                    pump()
                    pump()
                    pump()
                    kproj_cells(psP, [(0, 3)], wk_sb, xk_sl, kt_sb)
                    # first cells of K pair 1 / Q pair 1 must be issued
                    # before the prime pumps reach pair-1 logits (cursor can
                    # pass 16); the rest spread into the first steps as
                    # deadline-scheduled work items
                    kproj_cells(psP, [(1, 0)], wk_sb, xk_sl, kt_sb)
                    kproj_cells(psP, [(1, 0)], wq_sb, xq_sl, qt_sb)
                    kproj_cells(psP, [(1, 1)], wk_sb, xk_sl, kt_sb)
                    while cursor[0] < PREF:
                        pump()

                    # remaining Q cells, split into d-halves (one half per
                    # step keeps psl delivery timely)
                    qstate = [None]

                    def q_item(item):
                        (e, lb), half = item
                        if half == 0:
                            qstate[0] = psP.tile(
                                [128, 512], f32, tag="ph1", name=f"qc_{e}_{lb}"
                            )
                        ps = qstate[0]
                        for d in range(half * 4, half * 4 + 4):
                            nc.tensor.matmul(
                                ps[:],
                                wq_sb[:, d, e * 128 : (e + 1) * 128],
                                xq_sl(d, lb),
                                start=(d == 0),
                                stop=(d == DCH - 1),
                            )
                        if half == 1:
                            nc.vector.tensor_copy(
                                out=qt_sb[:, e, lb * 512 : (lb + 1) * 512],
                                in_=ps[:],
                            )

                    qitems = deque(
                        [
                            (c, h)
                            for c in [(0, 2), (1, 2), (0, 3), (1, 3)]
                            for h in range(2)
                        ]
                    )

                    # V projection one t-chunk per step (single psP slot)
                    def v_item(t):
                        ps = psP.tile([128, E], f32, tag="ph1", name=f"v_{t}")
                        for d in range(DCH):
                            nc.tensor.matmul(
                                ps[:],
                                xtv[:, t // 4, d, (t % 4) * 128 : (t % 4 + 1) * 128],
                                wv_sb[:, d, :],
                                start=(d == 0),
                                stop=(d == DCH - 1),
                            )
                        nc.vector.tensor_copy(
                            out=v_sb[:, t, :].rearrange(
                                "p (n c) -> p n c", n=HPC
                            )[:, :, 0:HDIM],
                            in_=ps[:].rearrange("p (n c) -> p n c", n=HPC),
                        )

                    vitems = deque(range(TCH))
                    # K pair-1 / Q cells with early-step deadlines
                    kq_items = deque(
                        [
                            ("k", (1, 2), None),
                            ("k", (1, 3), None),
                            ("q", (0, 1), 0),
                            ("q", (0, 1), 1),
                            ("q", (1, 1), 0),
                            ("q", (1, 1), 1),
                        ]
                    )

                    psoAB = [None, None]

                    for si, (fb, p, t) in enumerate(steps):
                        f0 = fb * FB
                        first_block = fb == 0 and p == 0
                        if t == 0:
                            psoAB[0] = psO.tile(
                                [VW, FB], f32, tag="pso", name=f"psoA_{fb}_{p}"
                            )
                            psoAB[1] = psO.tile(
                                [VW, FB], f32, tag="pso", name=f"psoB_{fb}_{p}"
                            )
                        if si % 2 == 0:
                            pump_logits()
                            pump_logits()
                        pump_exp()
                        if first_block and vitems:
                            v_item(vitems.popleft())
                        if first_block and t % 2 == 1 and kq_items:
                            kind, cell, half = kq_items.popleft()
                            if kind == "k":
                                kproj_cells(psP, [cell], wk_sb, xk_sl, kt_sb)
                            else:
                                q_item((cell, half))
                        if fb == 0 and p == 1 and qitems:
                            q_item(qitems.popleft())
                        et = etq.popleft()
                        for j in range(2):
                            h = 2 * p + j
                            nc.tensor.matmul(
                                psoAB[j][:],
                                v_sb[:, t, h * VW : (h + 1) * VW],
                                et[:, j, :],
                                start=(t == 0),
                                stop=(t == TCH - 1),
                            )
                        if t % 4 == 2 and pending:
                            outproj_group(*pending.pop(0))

                        if t == TCH - 1:
                            # normalize: O.T = O_un.T * (1/norm); both heads'
                            # norm rows share one DMA round trip via a fused
                            # stage tile [VW, 2, FB]
                            last_block = si == len(steps) - 1
                            stage = rpool.tile(
                                [VW, 2, FB],
                                f32,
                                tag="st",
                                name=f"stage_{fb}_{p}",
                            )
                            nc.vector.tensor_copy(
                                out=stage[:, 0, :], in_=psoAB[0][:]
                            )
                            if last_block:
                                nc.scalar.copy(
                                    out=stage[:, 1, :], in_=psoAB[1][:]
                                )
                            else:
                                nc.vector.tensor_copy(
                                    out=stage[:, 1, :], in_=psoAB[1][:]
                                )
                            if last_block:
                                # keep HAM warm through the normalize chain
                                wps2 = psL.tile(
                                    [128, 2, FB], f32, tag="psl", name="warm2"
                                )
                                for _ in range(12):
                                    nc.tensor.matmul(
                                        wps2[:, 0, :],
                                        warm_sb[:, 0:128],
                                        warm_sb[:],
                                        start=True,
                                        stop=True,
                                    )
                            dmae = nc.scalar if last_block else nc.sync
                            rsq = rpool.tile([32, FB // 16], f32, tag="rsq")
                            dmae.dma_start(
                                out=rsq[:],
                                in_=stage[HDIM : HDIM + 1, :, :],
                            )
                            rsf = rpool.tile([32, FB // 16], f32, tag="rsf")
                            with nc.allow_low_precision(
                                reason="softmax recip"
                            ):
                                nc.vector.reciprocal(out=rsf[:], in_=rsq[:])
                            rt2 = rpool.tile([1, 2, FB], f32, tag="rt2")
                            dmae.dma_start(out=rt2[:], in_=rsf[:])
                            for j in range(2):
                                if last_block:
                                    # matmul broadcast: psO slots are free to
                                    # hold (no next block), and it skips the
                                    # slow gpsimd+drain on the critical tail
                                    bt = psO.tile(
                                        [64, FB],
                                        f32,
                                        tag="pso",
                                        name=f"bt_{fb}_{p}_{j}",
                                    )
                                    nc.tensor.matmul(
                                        bt[:],
                                        ones_f32[0:1, :],
                                        rt2[0:1, j, :],
                                        start=True,
                                        stop=True,
                                    )
                                else:
                                    bt = bpool.tile(
                                        [64, FB], f32, tag=f"b{j}"
                                    )
                                    nc.gpsimd.partition_broadcast(
                                        bt[:], rt2[0:1, j, :], channels=64
                                    )
                                if j == 0:
                                    nc.vector.tensor_mul(
                                        out=ont[0:HDIM, p, f0 : f0 + FB],
                                        in0=stage[0:HDIM, 0, :],
                                        in1=bt[:],
                                    )
                                else:
                                    tmp = bpool.tile([64, FB], bf16, tag="tmp")
                                    nc.vector.tensor_mul(
                                        out=tmp[:],
                                        in0=stage[0:HDIM, 1, :],
                                        in1=bt[:],
                                    )
                                    dmae.dma_start(
                                        out=ont[64:128, p, f0 : f0 + FB],
                                        in_=tmp[:],
                                    )
                            if p == NP - 1:
                                pending += [
                                    (lc, db)
                                    for lc in range(
                                        fb * (FB // 128),
                                        (fb + 1) * (FB // 128),
                                    )
                                    for db in range(D // FB)
                                ]

                    # drain the last f-block's out-projection
                    for gi, (lc, db) in enumerate(pending):
                        outproj_group(lc, db, drain=gi + 1)
                    psW_holder[0][0].__exit__(None, None, None)
                    psO_cm.__exit__(None, None, None)

    nc.compile()
    return nc


def _get_nc():
    global _CACHED_NC
    if _CACHED_NC is None:
        _CACHED_NC = _build_nc()
    return _CACHED_NC


def _make_in_maps(query_input, key_input, value_input, Wq, Wk, Wv, Wo):
    import ml_dtypes

    bf16 = ml_dtypes.bfloat16
    scale = np.float32(HDIM) ** np.float32(-0.5)

    def panel_tile(x):
        # [D, L] -> [panel, partition, chunk, 512]
        return np.ascontiguousarray(
            x.T.reshape(DCH, 128, NFB, 512).transpose(2, 1, 0, 3)
        ).astype(bf16)

    xT = {}
    for b in range(B):
        xT[("q", b)] = panel_tile(query_input[b])
        xT[("k", b)] = panel_tile(key_input[b])
        xT[("v", b)] = panel_tile(value_input[b])

    in_maps = []
    for core in range(NCORES):
        b = core // 4
        g = core % 4
        hs = slice(g * HPC, (g + 1) * HPC)
        in_maps.append(
            {
                "xq": xT[("q", b)],
                "xk": xT[("k", b)],
                "xv": xT[("v", b)],
                "wq": np.ascontiguousarray(
                    (Wq[:, hs, :] * scale).reshape(D, E)
                ).astype(bf16),
                "wk": np.ascontiguousarray(Wk[:, hs, :].reshape(D, E)).astype(bf16),
                "wv": np.ascontiguousarray(Wv[:, hs, :].reshape(D, E)).astype(bf16),
                "wo": np.ascontiguousarray(Wo[hs].reshape(E, D)).astype(bf16),
            }
        )
    return in_maps


def _combine(results):
    out = np.empty((B, L, D), dtype=np.float32)
    for b in range(B):
        acc = results[b * 4]["out"].astype(np.float32)
        for g in range(1, 4):
            acc = acc + results[b * 4 + g]["out"]
        out[b] = acc
    return out


def kernel(query_input, key_input, value_input, Wq, Wk, Wv, Wo):
    from concourse.bass_utils import run_bass_kernel_spmd

    nc = _get_nc()
    in_maps = _make_in_maps(query_input, key_input, value_input, Wq, Wk, Wv, Wo)
    res = run_bass_kernel_spmd(nc, in_maps, core_ids=list(range(NCORES)))
    return _combine(res.results)


if __name__ == "__main__":
    rng = np.random.default_rng(0)
    inputs = {
        "query_input": rng.standard_normal((B, L, D), dtype=np.float32),
        "key_input": rng.standard_normal((B, L, D), dtype=np.float32),
        "value_input": rng.standard_normal((B, L, D), dtype=np.float32),
        "Wq": rng.standard_normal((D, NHEADS, HDIM), dtype=np.float32) * 0.03,
        "Wk": rng.standard_normal((D, NHEADS, HDIM), dtype=np.float32) * 0.03,
        "Wv": rng.standard_normal((D, NHEADS, HDIM), dtype=np.float32) * 0.03,
        "Wo": rng.standard_normal((NHEADS, HDIM, D), dtype=np.float32) * 0.03,
    }
    out = kernel(**inputs)
    print("kernel output", out.shape, out.dtype, float(np.abs(out).mean()))
